# revision 4
# baseline (speedup 1.0000x reference)
"""Luong concat attention on 8 Trainium2 NeuronCores (Bass/Tile, data-parallel over batch).

Shapes (hardcoded): query [1,32,1024] f32, values [32,2048,1024] f32,
W_w [1024,2048] f32, W_b [1024] f32, V_w [1,1024] f32, V_b [1] f32.
Returns (context_vector [32,1024] f32, attention_weights [32,2048,1] f32).

Strategy: shard batch B=32 over 8 cores (4 each). Host precomputes
qbias = q @ Wq.T + W_b (tiny) and the transposed/bf16 weight layouts.
Per batch on device:
  1. values tiles [128t,1024h] f32 -> bf16 -> xbar DMA-transpose to [h,t]
  2. pre[u,t] = WvT.T @ valT  (bf16 matmuls, fp32 PSUM)
  3. tanh(pre + qbias[u]) on ScalarE (bias per-partition), bf16 out
  4. score[1,t] = V.T @ tanh   (PE), exp on ScalarE with fused sum
  5. normalize -> attention weights out; PE-transpose e to [t,1] layout
  6. context[1,h] = e.T @ values (fp32r matmuls vs re-loaded f32 values)
"""
import numpy as np
import ml_dtypes
from contextlib import ExitStack

import concourse.bass as bass
import concourse.bacc as bacc
import concourse.tile as tile
import concourse.mybir as mybir
from concourse.bass_utils import run_bass_kernel_spmd

F32 = mybir.dt.float32
F32R = mybir.dt.float32r
BF16 = mybir.dt.bfloat16
AF = mybir.ActivationFunctionType

B, T, H, U = 32, 2048, 1024, 1024
N_CORES = 8
BC = B // N_CORES          # batches per core
NT = T // 128              # 16 t-tiles per batch
NH = H // 128              # 8 h-chunks
NU = U // 128              # 8 u-chunks
TC4 = 4                    # t-chunks of 512 per batch
_KERNEL_CACHE = {}


def build_nc(reps: int = 1):
    """Build + compile the per-core program (SPMD, identical on all cores)."""
    nc = bacc.Bacc("TRN2", target_bir_lowering=False, debug=False,
                   num_devices=N_CORES)
    values_d = nc.dram_tensor("values", [BC, T, H], F32, kind="ExternalInput").ap()
    qbias_d = nc.dram_tensor("qbias", [BC, 128, NU], F32, kind="ExternalInput").ap()
    wvt_d = nc.dram_tensor("wvt", [H, U], BF16, kind="ExternalInput").ap()
    vcol_d = nc.dram_tensor("vcol", [128, NU], BF16, kind="ExternalInput").ap()
    ctx_d = nc.dram_tensor("ctx", [BC, H], F32, kind="ExternalOutput").ap()
    attw_d = nc.dram_tensor("attw", [BC, T], F32, kind="ExternalOutput").ap()

    with tile.TileContext(nc) as tc, ExitStack() as ctx:
        const = ctx.enter_context(tc.tile_pool(name="const", bufs=1))
        qbp = ctx.enter_context(tc.tile_pool(name="qbp", bufs=2))
        vnp = ctx.enter_context(tc.tile_pool(name="vnp", bufs=4))
        vbp = ctx.enter_context(tc.tile_pool(name="vbp", bufs=4))
        vtp = ctx.enter_context(tc.tile_pool(name="vtp", bufs=2))
        thp = ctx.enter_context(tc.tile_pool(name="thp", bufs=10))
        efp = ctx.enter_context(tc.tile_pool(name="efp", bufs=2))
        smp = ctx.enter_context(tc.tile_pool(name="smp", bufs=2))
        vcp = ctx.enter_context(tc.tile_pool(name="vcp", bufs=4))
        vcr = ctx.enter_context(tc.tile_pool(name="vcr", bufs=4))
        cxp = ctx.enter_context(tc.tile_pool(name="cxp", bufs=2))
        pp_ps = ctx.enter_context(tc.tile_pool(name="pp_ps", bufs=2, space="PSUM"))
        sc_ps = ctx.enter_context(tc.tile_pool(name="sc_ps", bufs=2, space="PSUM"))
        et_ps = ctx.enter_context(tc.tile_pool(name="et_ps", bufs=1, space="PSUM"))
        cp_ps = ctx.enter_context(tc.tile_pool(name="cp_ps", bufs=1, space="PSUM"))

        # constants: weights WvT as [128p, k, u], V col as [128p, u-chunk], identity
        wvt_sb = const.tile([128, NH, U], BF16, tag="wvt")
        nc.sync.dma_start(wvt_sb[:], wvt_d.rearrange("(k p) u -> p k u", p=128))
        vcol_sb = const.tile([128, NU], BF16, tag="vcol")
        nc.sync.dma_start(vcol_sb[:], vcol_d[:])
        ident = const.tile([1, 1], F32, tag="ident")
        nc.vector.memset(ident[:], 1.0)

        for _ in range(reps):
            for b in range(BC):
                qb = qbp.tile([128, NU], F32, tag="qb")
                nc.sync.dma_start(qb[:], qbias_d[b])

                # load values natural, cast, xbar-transpose into vT
                vT = vtp.tile([128, NT, NH, 128], BF16, tag="vT")
                for i in range(NT):
                    vn = vnp.tile([128, H], F32, tag="vn")
                    nc.sync.dma_start(vn[:], values_d[b, i * 128:(i + 1) * 128, :])
                    vb_ = vbp.tile([128, H], BF16, tag="vb")
                    nc.vector.tensor_copy(vb_[:], vn[:])
                    nc.scalar.dma_start_transpose(vT[:, i, :, :], vb_[:])

                # pre matmuls + tanh eviction
                tanh_tiles = []
                for u in range(NU):
                    th = thp.tile([128, T], BF16, tag="th")
                    tanh_tiles.append(th)
                    for c in range(TC4):
                        pp = pp_ps.tile([128, 512], F32, tag="pp")
                        for k in range(NH):
                            nc.tensor.matmul(
                                pp[:],
                                wvt_sb[:, k, u * 128:(u + 1) * 128],
                                vT[:, 4 * c:4 * c + 4, k, :],
                                start=(k == 0), stop=(k == NH - 1),
                            )
                        nc.scalar.activation(
                            th[:, 512 * c:512 * (c + 1)], pp[:], AF.Tanh,
                            bias=qb[:, u:u + 1],
                        )

                # scores -> exp (with fused partial sums)
                ef = efp.tile([1, T], F32, tag="ef")
                zp = smp.tile([1, TC4], F32, tag="zp")
                for c in range(TC4):
                    sc = sc_ps.tile([1, 512], F32, tag="sc")
                    for u in range(NU):
                        nc.tensor.matmul(
                            sc[:], vcol_sb[:, u:u + 1],
                            tanh_tiles[u][:, 512 * c:512 * (c + 1)],
                            start=(u == 0), stop=(u == NU - 1),
                        )
                    nc.scalar.activation(ef[:, 512 * c:512 * (c + 1)], sc[:],
                                         AF.Exp, accum_out=zp[:, c:c + 1])

                # normalize in place; write attention weights
                z = smp.tile([1, 1], F32, tag="z")
                nc.vector.reduce_sum(z[:], zp[:], axis=mybir.AxisListType.X)
                rz = smp.tile([1, 1], F32, tag="rz")
                nc.vector.reciprocal(rz[:], z[:])
                nc.vector.tensor_scalar_mul(ef[:], ef[:], rz[:])
                nc.sync.dma_start(attw_d[b:b + 1, :], ef[:])

                # transpose normalized weights to [t, 1] layout
                etp = et_ps.tile([128, NT], F32, tag="etp")
                for j in range(NT):
                    nc.tensor.transpose(etp[:, j:j + 1],
                                        ef[:, 128 * j:128 * (j + 1)], ident[:])
                et = smp.tile([128, NT], F32R, tag="et")
                nc.vector.tensor_copy(et[:], etp[:])

                # context: reload values f32, round to f32r, matmuls into 2 PSUM halves
                cp0 = cp_ps.tile([1, 512], F32, tag="cp0")
                cp1 = cp_ps.tile([1, 512], F32, tag="cp1")
                for j in range(NT):
                    vc_raw = vcp.tile([128, H], F32, tag="vc")
                    nc.sync.dma_start(vc_raw[:], values_d[b, j * 128:(j + 1) * 128, :])
                    vc = vcr.tile([128, H], F32R, tag="vcr")
                    nc.vector.tensor_copy(vc[:], vc_raw[:])
                    lhs = et[:, j:j + 1]
                    nc.tensor.matmul(cp0[:], lhs, vc[:, 0:512],
                                     start=(j == 0), stop=(j == NT - 1))
                    nc.tensor.matmul(cp1[:], lhs, vc[:, 512:1024],
                                     start=(j == 0), stop=(j == NT - 1))
                cxs = cxp.tile([1, H], F32, tag="cxs")
                nc.vector.tensor_copy(cxs[:, 0:512], cp0[:])
                nc.vector.tensor_copy(cxs[:, 512:1024], cp1[:])
                nc.sync.dma_start(ctx_d[b:b + 1, :], cxs[:])

    nc.compile()
    return nc


def _get_nc(reps: int = 1):
    if reps not in _KERNEL_CACHE:
        _KERNEL_CACHE[reps] = build_nc(reps)
    return _KERNEL_CACHE[reps]


def host_prep(query, values, W_w, W_b, V_w, V_b):
    q = np.asarray(query, dtype=np.float32)[0]                     # [B, H]
    W = np.asarray(W_w, dtype=np.float32)
    qbias = q @ W[:, H:].T + np.asarray(W_b, dtype=np.float32)     # [B, U]
    qbias_r = np.ascontiguousarray(
        qbias.reshape(B, NU, 128).transpose(0, 2, 1))              # [B, 128, NU]
    wvt = np.ascontiguousarray(W[:, :H].T).astype(ml_dtypes.bfloat16)  # [H, U]
    vcol = np.ascontiguousarray(
        np.asarray(V_w, dtype=np.float32)[0].reshape(NU, 128).T
    ).astype(ml_dtypes.bfloat16)                                   # [128, NU]
    vals = np.asarray(values, dtype=np.float32)
    in_maps = []
    for c in range(N_CORES):
        sl = slice(c * BC, (c + 1) * BC)
        in_maps.append({
            "values": np.ascontiguousarray(vals[sl]),
            "qbias": np.ascontiguousarray(qbias_r[sl]),
            "wvt": wvt,
            "vcol": vcol,
        })
    return in_maps


def kernel(query, values, W_w, W_b, V_w, V_b):
    in_maps = host_prep(query, values, W_w, W_b, V_w, V_b)
    nc = _get_nc()
    res = run_bass_kernel_spmd(nc, in_maps, list(range(N_CORES)))
    ctxv = np.concatenate([res.results[c]["ctx"] for c in range(N_CORES)], axis=0)
    attw = np.concatenate([res.results[c]["attw"] for c in range(N_CORES)], axis=0)
    return ctxv.astype(np.float32), attw.reshape(B, T, 1).astype(np.float32)


if __name__ == "__main__":
    rng = np.random.default_rng(0)
    inputs = {
        "query": rng.standard_normal((1, B, H), dtype=np.float32),
        "values": rng.standard_normal((B, T, H), dtype=np.float32),
        "W_w": (rng.standard_normal((U, 2 * H), dtype=np.float32) / np.sqrt(2 * H)).astype(np.float32),
        "W_b": (rng.standard_normal(U, ).astype(np.float32) * 0.02),
        "V_w": (rng.standard_normal((1, U), dtype=np.float32) / np.sqrt(U)).astype(np.float32),
        "V_b": (rng.standard_normal(1).astype(np.float32) * 0.02),
    }
    ctxv, attw = kernel(**inputs)
    print("ctx", ctxv.shape, "attw", attw.shape)
